# revision 25
# baseline (speedup 1.0000x reference)
"""Trainium2 Bass kernel for the dual-stream position-aware GAT (EAGLE_V2).

Data-parallel over batch B=128 across 8 NeuronCores (16 batch elems/core).
Host pre-transposes h, pre-packs weights, and builds the 0/1 attention
masks (incl. the semantic top-K graph). The device program per batch
element runs 2 GAT layers per stream (syn/sem) with a fused
softmax-attention + LayerNorm+ReLU, then the fusion projection.

v2 rewrite vs baseline: eT scores built via one rank-1 matmul + ACT
Prelu-with-bias (fd as per-partition bias), multiplicative 0/1 mask on
DVE, softmax denominator/broadcast via tiny PE matmuls, LN rstd via ACT
Ln/Exp (one act table), zero GpSimd work, and PSUM split into dedicated
bank rings (Pmm/attA/hP) so batch elements pipeline.

Self-contained: hardcodes all shapes from the problem spec.
"""
import os
import sys

sys.path.insert(0, "/opt/trn_rl_repo")
os.environ.setdefault("MYCRO_LOCAL_CACHE", "1")

from contextlib import ExitStack

import ml_dtypes
import numpy as np

import concourse.bass as bass
import concourse.tile as tile
from concourse import bacc, mybir
from concourse.bass_utils import run_bass_kernel_spmd

B, N, H, G, TOPK = 128, 256, 768, 300, 10
NCORES = 8
BL = B // NCORES
LN_EPS = 1e-5
F32 = mybir.dt.float32
F32R = mybir.dt.float32r
I32 = mybir.dt.int32
BF16 = mybir.dt.bfloat16
BF = ml_dtypes.bfloat16

KC0 = H // 128  # 6 K-chunks for the H contraction
# L1 / fusion contraction chunks over G=300: 128, 128, 44
GCH = [(0, 128), (128, 128), (256, 44)]

_prog_cache = {}


def _build_program(n_b, pos_per_b, has_tb, has_ln, has_fusb, repeat=1):
    nc = bacc.Bacc("TRN2", target_bir_lowering=False, debug=False)

    d = {}
    d["hT"] = nc.dram_tensor("hT", [n_b, H, N], F32R, kind="ExternalInput").ap()
    # 0/1 masks in eT orientation: mask[j, i] = adj[i, j] > 0
    d["negms"] = nc.dram_tensor("negms", [n_b, N, N], BF16, kind="ExternalInput").ap()
    d["negmm"] = nc.dram_tensor("negmm", [n_b, N, N], BF16, kind="ExternalInput").ap()
    d["w0"] = nc.dram_tensor("w0", [H, 1204], F32R, kind="ExternalInput").ap()
    np0 = n_b if pos_per_b else 1
    d["pos0"] = nc.dram_tensor("pos0", [np0, N, 1204], F32R, kind="ExternalInput").ap()
    d["w1"] = nc.dram_tensor("w1", [128, 3, 604], BF16, kind="ExternalInput").ap()
    d["pos1"] = nc.dram_tensor("pos1", [np0, N, 604], BF16, kind="ExternalInput").ap()
    d["fusw"] = nc.dram_tensor("fusw", [128, 6, G], BF16, kind="ExternalInput").ap()
    d["fusb"] = nc.dram_tensor("fusb", [1, G], BF16, kind="ExternalInput").ap()
    d["i128f"] = nc.dram_tensor("i128f", [128, 128], F32R, kind="ExternalInput").ap()
    d["i128b"] = nc.dram_tensor("i128b", [128, 128], BF16, kind="ExternalInput").ap()
    if has_ln:
        d["lng"] = nc.dram_tensor("lng", [128, 4, G], F32, kind="ExternalInput").ap()
        d["lnb"] = nc.dram_tensor("lnb", [128, 4, G], F32, kind="ExternalInput").ap()
    out_d = nc.dram_tensor("out", [n_b, N, G], F32, kind="ExternalOutput").ap()

    with tile.TileContext(nc) as tc, ExitStack() as ctx:
        cons = ctx.enter_context(tc.tile_pool(name="cons", bufs=1))
        sb = ctx.enter_context(tc.tile_pool(name="sb", bufs=3))
        ps = ctx.enter_context(tc.tile_pool(name="ps", bufs=2, space="PSUM"))

        # ---- constants / weights (loaded once) ----
        w0 = cons.tile([128, KC0, 1204], F32R, tag="w0")
        nc.sync.dma_start(w0[:], d["w0"].rearrange("(k p) c -> p k c", p=128))
        w1 = cons.tile([128, 3, 604], BF16, tag="w1")
        nc.sync.dma_start(w1[:], d["w1"])
        fusw = cons.tile([128, 6, G], BF16, tag="fusw")
        nc.sync.dma_start(fusw[:], d["fusw"])
        fusb = cons.tile([1, G], BF16, tag="fusb")
        nc.sync.dma_start(fusb[:], d["fusb"])
        i128f = cons.tile([128, 128], F32R, tag="i128f")
        nc.sync.dma_start(i128f[:], d["i128f"])
        i128b = cons.tile([128, 128], BF16, tag="i128b")
        nc.sync.dma_start(i128b[:], d["i128b"])
        i128ff = cons.tile([128, 128], F32, tag="i128ff")
        nc.sync.dma_start(i128ff[:], d["i128f"].bitcast(F32))
        onescol = cons.tile([128, 1], BF16, tag="onescol")
        nc.vector.memset(onescol[:], 1.0)
        onesrow_bf = cons.tile([1, N], BF16, tag="onesrow_bf")
        nc.vector.memset(onesrow_bf[:], 1.0)
        onesrow_f = cons.tile([1, 128], F32, tag="onesrow_f")
        nc.vector.memset(onesrow_f[:], 1.0)
        if not pos_per_b:
            pos0 = cons.tile([128, 2, 1204], F32R, tag="pos0")
            nc.sync.dma_start(pos0[:], d["pos0"][0].rearrange("(m p) c -> p m c", p=128))
            pos1 = cons.tile([128, 2, 604], BF16, tag="pos1")
            nc.sync.dma_start(pos1[:], d["pos1"][0].rearrange("(m p) c -> p m c", p=128))
        if has_ln:
            lng = cons.tile([128, 4, G], F32, tag="lng")
            nc.sync.dma_start(lng[:], d["lng"])
            lnb = cons.tile([128, 4, G], F32, tag="lnb")
            nc.sync.dma_start(lnb[:], d["lnb"])

        AF = mybir.ActivationFunctionType
        OP = mybir.AluOpType

        def attention(sl_idx, whsb, fsrow, s, fdsb, mask, seed, bst4, bag4, slot):
            """softmax-attention for one stream-layer, through bn stats.

            whsb: sbuf bf16 [128, 2, 300] (Wh for this stream)
            fsrow: sbuf f32 [1, 512] (fs rows, s-major)
            fdsb: sbuf f32 [128, 2, 4-ish] holding fd columns at channel 2s
            mask: sbuf bf16 [128, 2, 256] 0/1 mask (eT orientation)
            seed(im, hP): emits residual-seeding matmuls into hP
                 (start=True ... stop=False); h' accumulates on top.
            bst4/bag4: [128, 4, 6]/[128, 4, 2] shared LN stats tiles; this
                 stream writes lanes slot*2 + im.
            returns hP psum APs [im] (LN+relu consumed later by ln_tail).
            """
            lr = sb.tile([128, 2, 256], F32, tag="lr", name="lr", bufs=6)
            for jm in range(2):
                nc.scalar.activation(
                    lr[:, jm, :], fsrow[:, 256 * s : 256 * (s + 1)], AF.Prelu,
                    bias=fdsb[:, jm, 2 * s : 2 * s + 1], alpha=0.2,
                )
            num = sb.tile([128, 2, 256], BF16, tag="num", name="num", bufs=6)
            nc.scalar.activation(num[:], lr[:], AF.Exp)
            numm = sb.tile([128, 2, 256], BF16, tag="numm", name="numm", bufs=6)
            nc.vector.tensor_mul(numm[:], num[:], mask[:])

            sRt = ps.tile([1, 256], F32, tag="SM", bufs=2, name=f"sR{sl_idx}")
            for jm in range(2):
                nc.tensor.matmul(
                    sRt[:], onescol[:], numm[:, jm, :], start=(jm == 0), stop=(jm == 1)
                )
            rr = sb.tile([1, 256], F32, tag="rr", name="rr", bufs=6)
            nc.vector.reciprocal_approx_fast(rr[:], sRt[:])
            recb = sb.tile([128, 256], F32, tag="recb", name="recb", bufs=6)
            nc.gpsimd.partition_broadcast(recb[:], rr[:])
            num_m = sb.tile([128, 2, 256], BF16, tag="num_m", name="num_m", bufs=6)
            for jm in range(2):
                nc.vector.tensor_mul(num_m[:, jm, :], numm[:, jm, :], recb[:])

            hPs = []
            for im in range(2):
                hPt = ps.tile([128, G], F32, tag="hP", bufs=4, name=f"hP{sl_idx}_{im}")
                hP = hPt[:]
                hPs.append(hP)
                seed(im, hP)
                for jm in range(2):
                    nc.tensor.matmul(
                        hP,
                        num_m[:, jm, 128 * im : 128 * (im + 1)],
                        whsb[:, jm, 0:G],
                        start=False,
                        stop=(jm == 1),
                    )
                k = 2 * slot + im
                nc.vector.bn_stats(bst4[:, k, :], hP)
                nc.vector.bn_aggr(bag4[:, k, :], bst4[:, k, :])
            return hPs

        def quake_rsqrt(u, x, k):
            """x = 1/sqrt(u) via Quake seed + 2 Newton iterations. [128,k] f32."""
            MAGIC = 0x5F3759DF
            t0 = sb.tile([128, k], F32, tag="rsq_t0", name="rsq_t0")
            nc.vector.tensor_scalar(
                t0[:].bitcast(I32), u.bitcast(I32), 1, None, OP.arith_shift_right
            )
            nc.vector.tensor_scalar(
                x.bitcast(I32), t0[:].bitcast(I32), MAGIC, -1, OP.subtract, OP.mult
            )
            for _ in range(2):
                sq = sb.tile([128, k], F32, tag="rsq_sq", name="rsq_sq")
                nc.vector.tensor_mul(sq[:], x, x)
                t = sb.tile([128, k], F32, tag="rsq_t", name="rsq_t")
                nc.vector.scalar_tensor_tensor(t[:], sq[:], 0.5, u, OP.mult, OP.mult)
                nc.vector.tensor_scalar(t[:], t[:], -1.0, 1.5, OP.mult, OP.add)
                nc.vector.tensor_mul(x, x, t[:])

        def ln_tail(layer, hPs_by_slot, bag4, sl_idxs, out_tags):
            """LN+relu tail for one or more stream slots of a layer.

            hPs_by_slot: {slot: [hP_im0, hP_im1]}; bag4 [128, 2*nslots, 2].
            Returns {slot: y sbuf bf16 [128, 2, 300]}.
            """
            slots = sorted(hPs_by_slot.keys())
            nk = 2 * len(slots)
            tsuf = f"{layer}_{sl_idxs[0]}"
            u = sb.tile([128, nk], F32, tag=f"u{tsuf}", name="u")
            nc.vector.tensor_scalar(u[:], bag4[:, :, 1], LN_EPS, None, OP.add)
            rstd = sb.tile([128, nk], F32, tag=f"rstd{tsuf}", name="rstd")
            quake_rsqrt(u[:], rstd[:], nk)
            nmr = sb.tile([128, nk], F32, tag=f"nmr{tsuf}", name="nmr")
            nc.vector.scalar_tensor_tensor(
                nmr[:], bag4[:, :, 0], -1.0, rstd[:], OP.mult, OP.mult
            )
            ys = {}
            for slot in slots:
                sl_idx = sl_idxs[slot]
                y = sb.tile([128, 2, G], BF16, tag=out_tags[slot], name=out_tags[slot])
                ys[slot] = y
                for im in range(2):
                    k = 2 * slot + im
                    if has_ln:
                        xn = sb.tile([128, G], F32, tag="xn", name="xn")
                        nc.scalar.activation(
                            xn[:], hPs_by_slot[slot][im], AF.Identity,
                            bias=nmr[:, k : k + 1], scale=rstd[:, k : k + 1],
                        )
                        xg = sb.tile([128, G], F32, tag="xg", name="xg")
                        nc.vector.scalar_tensor_tensor(
                            xg[:], xn[:], 1.0, lng[:, sl_idx, :], OP.mult, OP.mult
                        )
                        nc.vector.tensor_add(xg[:], xg[:], lnb[:, sl_idx, :])
                        nc.vector.tensor_scalar(y[:, im, :], xg[:], 0.0, None, OP.max)
                    else:
                        nc.scalar.activation(
                            y[:, im, :], hPs_by_slot[slot][im], AF.Relu,
                            bias=nmr[:, k : k + 1], scale=rstd[:, k : k + 1],
                        )
            return ys

        def transpose_y(y, tag):
            """y sbuf bf16 [128,2,300] -> yT sbuf bf16 [128,3,256] (K chunks)."""
            yT = sb.tile([128, 3, N], BF16, tag=tag, name=tag)
            for ci, (c0, cw) in enumerate(GCH):
                yTp = ps.tile([128, N], BF16, tag="PG", bufs=2, name="yTp")
                for im in range(2):
                    nc.tensor.transpose(
                        yTp[0:cw, 128 * im : 128 * (im + 1)],
                        y[:, im, c0 : c0 + cw],
                        i128b[:],
                    )
                if ci % 2 == 0:
                    nc.vector.tensor_copy(yT[0:cw, ci, :], yTp[0:cw, :])
                else:
                    nc.scalar.copy(yT[0:cw, ci, :], yTp[0:cw, :])
            return yT

        def fs_transposes(pe, name, ptag):
            """pe: sbuf f32 [128, 2, 4] (cols: synfd, synfs, semfd, semfs).
            Returns sbuf bf16 [128, 2, 256]: fs rows broadcast across
            partitions (dim 1 = stream)."""
            fsrowP = ps.tile([1, 512], F32, tag=ptag, name=f"fsP_{name}")
            for s in range(2):
                for m in range(2):
                    o = 256 * s + 128 * m
                    nc.tensor.transpose(
                        fsrowP[0:1, o : o + 128],
                        pe[:, m, 2 * s + 1 : 2 * s + 2],
                        i128ff[:],
                    )
            fsrow = sb.tile([1, 512], BF16, tag=f"fsrow_{name}", name=f"fsrow_{name}")
            nc.vector.tensor_copy(fsrow[:], fsrowP[:])
            fs_bc = sb.tile([128, 512], BF16, tag=f"fsbc_{name}", name=f"fsbc_{name}")
            nc.gpsimd.partition_broadcast(fs_bc[:], fsrow[:])
            return fs_bc

        # ================= per batch element =================
        # Software-pipelined: stage A(b) = input DMAs + L0 GEMM + fs rows
        # (depends only on inputs); stage B(b) = everything downstream.
        # A(b+1) is emitted before B(b) so the scheduler has independent
        # PE work during B's serial softmax/LN/transpose chains.
        def stage_a(b):
            pb = b if pos_per_b else 0
            if pos_per_b:
                pos0l = sb.tile([128, 2, 1204], F32R, tag="pos0b", bufs=4)
                nc.sync.dma_start(
                    pos0l[:], d["pos0"][pb].rearrange("(m p) c -> p m c", p=128)
                )
                pos1l = sb.tile([128, 2, 604], BF16, tag="pos1b", bufs=4)
                nc.sync.dma_start(
                    pos1l[:], d["pos1"][pb].rearrange("(m p) c -> p m c", p=128)
                )
            else:
                pos0l, pos1l = pos0, pos1

            hT = sb.tile([128, KC0, N], F32R, tag="hT", name="hT", bufs=4)
            nc.sync.dma_start(hT[:], d["hT"][b].rearrange("(k p) n -> p k n", p=128))
            m01_syn = sb.tile([128, 2, N], BF16, tag="m01_syn", name="m01_syn", bufs=4)
            nc.sync.dma_start(
                m01_syn[:], d["negms"][b].rearrange("(m p) n -> p m n", p=128)
            )
            m01_sem = sb.tile([128, 2, N], BF16, tag="m01_sem", name="m01_sem", bufs=4)
            nc.sync.dma_start(
                m01_sem[:], d["negmm"][b].rearrange("(m p) n -> p m n", p=128)
            )

            # ---- layer 0: both streams' Wh / fs/fd scores in one pass ----
            # w0 cols: [synW 0:300 | semW 300:600 | syn_tW 600:900 | sem_tW 900:1200
            #           | synfd, synfs, semfd, semfs 1200:1204]
            whsb0 = {}
            pe_sb = sb.tile([128, 2, 4], F32, tag="pe_sb", name="pe_sb", bufs=4)
            for s in range(2):
                whsb0[s] = sb.tile(
                    [128, 2, G], BF16, tag=f"whsb0_{s}", name=f"whsb0_{s}", bufs=4
                )
            for m in range(2):
                for s in range(2):
                    c0 = 302 * s
                    P0 = ps.tile([128, 302], F32, tag="PG", bufs=2, name="P0")
                    for k in range(KC0):
                        nc.tensor.matmul(
                            P0[:],
                            hT[:, k, 128 * m : 128 * (m + 1)],
                            w0[:, k, c0 : c0 + 302],
                            start=(k == 0),
                            stop=False,
                        )
                    nc.tensor.matmul(
                        P0[:],
                        i128f[:],
                        pos0l[:, m, c0 : c0 + 302],
                        start=False,
                        stop=True,
                    )
                    nc.scalar.copy(whsb0[s][:, m, :], P0[:, 0:300])
                    nc.scalar.copy(pe_sb[:, m, 2 * s : 2 * s + 2], P0[:, 300:302])

            fsrow0 = fs_transposes(pe_sb, "l0", "SM")
            return dict(
                hT=hT, m01_syn=m01_syn, m01_sem=m01_sem, whsb0=whsb0,
                pe_sb=pe_sb, fsrow0=fsrow0, pos0l=pos0l, pos1l=pos1l,
            )

        def stage_b(b, cx):
            hT = cx["hT"]
            m01_syn, m01_sem = cx["m01_syn"], cx["m01_sem"]
            whsb0, pe_sb, fsrow0 = cx["whsb0"], cx["pe_sb"], cx["fsrow0"]
            pos0l, pos1l = cx["pos0l"], cx["pos1l"]

            def seed_l0(s):
                def seed(im, hP):
                    c0 = 604 + s * G
                    for k in range(KC0):
                        nc.tensor.matmul(
                            hP,
                            hT[:, k, 128 * im : 128 * (im + 1)],
                            w0[:, k, c0 : c0 + G],
                            start=(k == 0),
                            stop=False,
                        )
                    if has_tb:
                        nc.tensor.matmul(
                            hP, i128f[:], pos0l[:, im, c0 : c0 + G],
                            start=False, stop=False,
                        )
                return seed

            ys = {}
            for s, mask in ((0, m01_syn), (1, m01_sem)):
                bst0 = sb.tile([128, 2, 6], F32, tag=f"bst0_{s}", name=f"bst0_{s}")
                bag0 = sb.tile([128, 2, 2], F32, tag=f"bag0_{s}", name=f"bag0_{s}")
                hp = attention(
                    s, whsb0[s], fsrow0, s, pe_sb, mask, seed_l0(s), bst0, bag0, 0
                )
                ys[s] = ln_tail(0, {0: hp}, bag0, (s,), (f"y{s}",))[0]

            # ---- layer 1 per stream ----
            # w1 cols: [synW1 0:300 | semW1 300:600 | synfd,synfs,semfd,semfs 600:604]
            y1 = {}
            yT1 = {}
            pe_sb1 = sb.tile([128, 2, 4], F32, tag="pe_sb1", name="pe_sb1")
            whsb1 = {}
            yTs = {}
            for s in range(2):
                yTs[s] = transpose_y(ys[s], f"yT0_{s}")
            for s in range(2):
                yT = yTs[s]
                whsb1[s] = sb.tile([128, 2, G], BF16, tag=f"whsb1_{s}", name=f"whsb1_{s}")
                for m in range(2):
                    c0 = 302 * s
                    P1 = ps.tile([128, 302], F32, tag="PG", bufs=2, name="P1")
                    for ki, (k0, kw) in enumerate(GCH):
                        nc.tensor.matmul(
                            P1[:],
                            yT[0:kw, ki, 128 * m : 128 * (m + 1)],
                            w1[0:kw, ki, c0 : c0 + 302],
                            start=(ki == 0),
                            stop=False,
                        )
                    nc.tensor.matmul(
                        P1[:],
                        i128b[:],
                        pos1l[:, m, c0 : c0 + 302],
                        start=False,
                        stop=True,
                    )
                    nc.vector.tensor_copy(whsb1[s][:, m, :], P1[:, 0:300])
                    nc.scalar.copy(pe_sb1[:, m, 2 * s : 2 * s + 2], P1[:, 300:302])
            fsrow1 = fs_transposes(pe_sb1, "l1", "SM")
            y1 = {}
            for s, mask in ((0, m01_syn), (1, m01_sem)):
                def seed_l1(im, hP, s=s):
                    nc.tensor.matmul(
                        hP, i128b[:], ys[s][:, im, :], start=True, stop=False
                    )
                bst1 = sb.tile([128, 2, 6], F32, tag=f"bst1_{s}", name=f"bst1_{s}")
                bag1 = sb.tile([128, 2, 2], F32, tag=f"bag1_{s}", name=f"bag1_{s}")
                hp = attention(
                    2 + s, whsb1[s], fsrow1, s, pe_sb1, mask, seed_l1, bst1, bag1, 0
                )
                y1[s] = ln_tail(1, {0: hp}, bag1, (2 + s,), (f"y{2+s}",))[0]
                yT1[s] = transpose_y(y1[s], f"yT1_{s}")

            # ---- fusion ----
            outsb = sb.tile([128, 2, G], F32, tag="outsb", name="outsb")
            for m in range(2):
                fP = ps.tile([128, G], F32, tag="PG", bufs=2, name="fP")
                first = True
                for s in range(2):
                    for ki, (k0, kw) in enumerate(GCH):
                        last = (s == 1 and ki == 2 and not has_fusb)
                        nc.tensor.matmul(
                            fP[:],
                            yT1[s][0:kw, ki, 128 * m : 128 * (m + 1)],
                            fusw[0:kw, 3 * s + ki, :],
                            start=first,
                            stop=last,
                        )
                        first = False
                if has_fusb:
                    nc.tensor.matmul(
                        fP[:],
                        onesrow_bf[0:1, 0:128],
                        fusb[:],
                        start=False,
                        stop=True,
                    )
                nc.scalar.activation(outsb[:, m, :], fP[:], AF.Relu)
            nc.sync.dma_start(
                out_d[b].rearrange("(m p) c -> p m c", p=128), outsb[:]
            )

        loop_ctx = tc.For_i(0, repeat, 1) if repeat > 1 else None
        if loop_ctx is not None:
            loop_ctx.__enter__()
        cx_next = stage_a(0)
        for b in range(n_b):
            cx = cx_next
            if b + 1 < n_b:
                cx_next = stage_a(b + 1)
            stage_b(b, cx)
        if loop_ctx is not None:
            loop_ctx.__exit__(None, None, None)

    nc.compile()
    return nc


def _host_pack(inputs):
    """Build all host-side arrays. Returns (per-core list of dicts, flags)."""
    h = np.asarray(inputs["h"], np.float32)
    adj = np.asarray(inputs["syntactic_adj"], np.float32)
    positions = np.asarray(inputs["positions"])

    hT = np.ascontiguousarray(h.transpose(0, 2, 1))
    # semantic graph mask on host (exact fp32, matches jax top_k tie-breaking)
    nrm = np.linalg.norm(h, axis=2, keepdims=True)
    hn = h / np.maximum(nrm, 1e-12)
    sim = np.matmul(hn, hn.transpose(0, 2, 1))  # [B,N,N] fp32
    order = np.argsort(-sim, axis=2, kind="stable")[:, :, :TOPK]
    maskA = np.zeros((h.shape[0], N, N), np.bool_)
    np.put_along_axis(maskA, order, True, axis=2)
    masksym = maskA | maskA.transpose(0, 2, 1)
    masksym |= np.eye(N, dtype=np.bool_)[None]  # reference adds +I unconditionally
    # 0/1 multiplicative masks, eT orientation (mask[j, i] = adj[i, j] > 0).
    # masksym is symmetric so no transpose needed for the semantic stream.
    negmm = masksym.astype(BF)
    negms = (adj.transpose(0, 2, 1) > 0).astype(BF)

    pos_same = bool((positions == positions[0:1]).all())
    pidx = positions[0] if pos_same else positions  # [N] or [B,N]

    def pack0(s):
        W = np.asarray(inputs[f"{s}0_W"], np.float64)
        asrc = np.asarray(inputs[f"{s}0_asrc"], np.float64)
        adst = np.asarray(inputs[f"{s}0_adst"], np.float64)
        return W, W @ adst, W @ asrc

    w0 = np.zeros((H, 1204), np.float64)
    pos_tabs0 = {}
    for si, s in enumerate(("syn", "sem")):
        W, wfd, wfs = pack0(s)
        w0[:, 302 * si : 302 * si + G] = W
        w0[:, 604 + si * G : 604 + (si + 1) * G] = np.asarray(inputs[f"{s}0_tW"], np.float64)
        w0[:, 302 * si + 300] = wfd
        w0[:, 302 * si + 301] = wfs
        pt = np.asarray(inputs[f"{s}0_pos"], np.float64)
        asrc = np.asarray(inputs[f"{s}0_asrc"], np.float64)
        adst = np.asarray(inputs[f"{s}0_adst"], np.float64)
        pos_tabs0[s] = (pt, pt @ adst, pt @ asrc)

    tb_syn = np.asarray(inputs["syn0_tb"], np.float64)
    tb_sem = np.asarray(inputs["sem0_tb"], np.float64)
    has_tb = bool(np.abs(tb_syn).max() > 0 or np.abs(tb_sem).max() > 0)

    def build_pos0(pidx1):  # pidx1: [N] int
        p = np.zeros((N, 1204), np.float64)
        for si, s in enumerate(("syn", "sem")):
            pt, pfd, pfs = pos_tabs0[s]
            p[:, 302 * si : 302 * si + G] = pt[pidx1]
            p[:, 302 * si + 300] = pfd[pidx1]
            p[:, 302 * si + 301] = pfs[pidx1]
        if has_tb:
            p[:, 604:904] = tb_syn[None, :]
            p[:, 904:1204] = tb_sem[None, :]
        return p

    w1 = np.zeros((384, 604), np.float64)
    pos_tabs1 = {}
    for si, s in enumerate(("syn", "sem")):
        W = np.asarray(inputs[f"{s}1_W"], np.float64)
        asrc = np.asarray(inputs[f"{s}1_asrc"], np.float64)
        adst = np.asarray(inputs[f"{s}1_adst"], np.float64)
        w1[:G, 302 * si : 302 * si + G] = W
        w1[:G, 302 * si + 300] = W @ adst
        w1[:G, 302 * si + 301] = W @ asrc
        pt = np.asarray(inputs[f"{s}1_pos"], np.float64)
        pos_tabs1[s] = (pt, pt @ adst, pt @ asrc)

    def build_pos1(pidx1):
        p = np.zeros((N, 604), np.float64)
        for si, s in enumerate(("syn", "sem")):
            pt, pfd, pfs = pos_tabs1[s]
            p[:, 302 * si : 302 * si + G] = pt[pidx1]
            p[:, 302 * si + 300] = pfd[pidx1]
            p[:, 302 * si + 301] = pfs[pidx1]
        return p

    # w1 pre-chunked to [128, 3, 604]
    w1c = np.zeros((128, 3, 604), np.float64)
    for ki, (k0, kw) in enumerate(GCH):
        w1c[:kw, ki, :] = w1[k0 : k0 + kw, :]

    fw = np.asarray(inputs["fus_W"], np.float64)  # [600, 300]
    fusw = np.zeros((128, 6, G), np.float64)
    for s in range(2):
        for ki, (k0, kw) in enumerate(GCH):
            fusw[:kw, 3 * s + ki, :] = fw[300 * s + k0 : 300 * s + k0 + kw, :]
    fusb = np.asarray(inputs["fus_b"], np.float64)[None, :]
    has_fusb = bool(np.abs(fusb).max() > 0)

    lngs = [np.asarray(inputs[k], np.float32) for k in ("syn0_lng", "sem0_lng", "syn1_lng", "sem1_lng")]
    lnbs = [np.asarray(inputs[k], np.float32) for k in ("syn0_lnb", "sem0_lnb", "syn1_lnb", "sem1_lnb")]
    has_ln = bool(
        any(np.abs(g - 1.0).max() > 0 for g in lngs) or any(np.abs(bb).max() > 0 for bb in lnbs)
    )

    shared = {
        "w0": w0.astype(np.float32),
        "w1": w1c.astype(BF),
        "fusw": fusw.astype(BF),
        "fusb": fusb.astype(BF),
        "i128f": np.eye(128, dtype=np.float32),
        "i128b": np.eye(128).astype(BF),
    }
    if has_ln:
        shared["lng"] = np.stack(
            [np.broadcast_to(g, (128, G)) for g in lngs], axis=1
        ).astype(np.float32).copy()
        shared["lnb"] = np.stack(
            [np.broadcast_to(bb, (128, G)) for bb in lnbs], axis=1
        ).astype(np.float32).copy()

    if pos_same:
        shared["pos0"] = build_pos0(pidx)[None].astype(np.float32)
        shared["pos1"] = build_pos1(pidx)[None].astype(BF)
        pos_per_b = False
    else:
        pos_per_b = True

    in_maps = []
    for c in range(NCORES):
        sl = slice(c * BL, (c + 1) * BL)
        m = dict(shared)
        m["hT"] = hT[sl]
        m["negms"] = negms[sl]
        m["negmm"] = negmm[sl]
        if pos_per_b:
            m["pos0"] = np.stack([build_pos0(positions[i]) for i in range(c * BL, (c + 1) * BL)]).astype(np.float32)
            m["pos1"] = np.stack([build_pos1(positions[i]) for i in range(c * BL, (c + 1) * BL)]).astype(BF)
        in_maps.append(m)

    flags = (BL, pos_per_b, has_tb, has_ln, has_fusb)
    return in_maps, flags


def _get_program(flags):
    if flags not in _prog_cache:
        _prog_cache[flags] = _build_program(*flags)
    return _prog_cache[flags]


_last_results = {}


def kernel(**inputs):
    in_maps, flags = _host_pack(inputs)
    nc = _get_program(flags)
    res = run_bass_kernel_spmd(nc, in_maps, list(range(NCORES)))
    _last_results["res"] = res
    out = np.concatenate([res.results[c]["out"] for c in range(NCORES)], axis=0)
    return np.ascontiguousarray(out.astype(np.float32))


# revision 26
# speedup vs baseline: 1.2405x; 1.2405x over previous
"""Trainium2 Bass kernel for the dual-stream position-aware GAT (EAGLE_V2).

Data-parallel over batch B=128 across 8 NeuronCores (16 batch elems/core).
Host pre-transposes h, pre-packs weights, and builds the 0/1 attention
masks (incl. the semantic top-K graph). The device program per batch
element runs 2 GAT layers per stream (syn/sem) with a fused
softmax-attention + LayerNorm+ReLU, then the fusion projection.

v2 rewrite vs baseline: eT scores built via one rank-1 matmul + ACT
Prelu-with-bias (fd as per-partition bias), multiplicative 0/1 mask on
DVE, softmax denominator/broadcast via tiny PE matmuls, LN rstd via ACT
Ln/Exp (one act table), zero GpSimd work, and PSUM split into dedicated
bank rings (Pmm/attA/hP) so batch elements pipeline.

Self-contained: hardcodes all shapes from the problem spec.
"""
import os
import sys

sys.path.insert(0, "/opt/trn_rl_repo")
os.environ.setdefault("MYCRO_LOCAL_CACHE", "1")

from contextlib import ExitStack

import ml_dtypes
import numpy as np

import concourse.bass as bass
import concourse.tile as tile
from concourse import bacc, mybir
from concourse.bass_utils import run_bass_kernel_spmd

B, N, H, G, TOPK = 128, 256, 768, 300, 10
NCORES = 8
BL = B // NCORES
LN_EPS = 1e-5
F32 = mybir.dt.float32
F32R = mybir.dt.float32r
I32 = mybir.dt.int32
BF16 = mybir.dt.bfloat16
BF = ml_dtypes.bfloat16

KC0 = H // 128  # 6 K-chunks for the H contraction
# L1 / fusion contraction chunks over G=300: 128, 128, 44
GCH = [(0, 128), (128, 128), (256, 44)]

_prog_cache = {}


def _build_program(n_b, pos_per_b, has_tb, has_ln, has_fusb, repeat=1):
    nc = bacc.Bacc("TRN2", target_bir_lowering=False, debug=False)

    d = {}
    d["hT"] = nc.dram_tensor("hT", [n_b, H, N], BF16, kind="ExternalInput").ap()
    # 0/1 masks in eT orientation: mask[j, i] = adj[i, j] > 0
    d["negms"] = nc.dram_tensor("negms", [n_b, N, N], BF16, kind="ExternalInput").ap()
    d["negmm"] = nc.dram_tensor("negmm", [n_b, N, N], BF16, kind="ExternalInput").ap()
    d["w0"] = nc.dram_tensor("w0", [H, 1204], BF16, kind="ExternalInput").ap()
    np0 = n_b if pos_per_b else 1
    d["pos0"] = nc.dram_tensor("pos0", [np0, N, 1204], F32R, kind="ExternalInput").ap()
    d["w1"] = nc.dram_tensor("w1", [128, 3, 604], BF16, kind="ExternalInput").ap()
    d["pos1"] = nc.dram_tensor("pos1", [np0, N, 604], BF16, kind="ExternalInput").ap()
    d["fusw"] = nc.dram_tensor("fusw", [128, 6, G], BF16, kind="ExternalInput").ap()
    d["fusb"] = nc.dram_tensor("fusb", [1, G], BF16, kind="ExternalInput").ap()
    d["i128f"] = nc.dram_tensor("i128f", [128, 128], F32R, kind="ExternalInput").ap()
    d["i128b"] = nc.dram_tensor("i128b", [128, 128], BF16, kind="ExternalInput").ap()
    if has_ln:
        d["lng"] = nc.dram_tensor("lng", [128, 4, G], F32, kind="ExternalInput").ap()
        d["lnb"] = nc.dram_tensor("lnb", [128, 4, G], F32, kind="ExternalInput").ap()
    out_d = nc.dram_tensor("out", [n_b, N, G], F32, kind="ExternalOutput").ap()

    with tile.TileContext(nc) as tc, ExitStack() as ctx:
        cons = ctx.enter_context(tc.tile_pool(name="cons", bufs=1))
        sb = ctx.enter_context(tc.tile_pool(name="sb", bufs=3))
        ps = ctx.enter_context(tc.tile_pool(name="ps", bufs=2, space="PSUM"))

        # ---- constants / weights (loaded once) ----
        w0 = cons.tile([128, KC0, 1204], BF16, tag="w0")
        nc.sync.dma_start(w0[:], d["w0"].rearrange("(k p) c -> p k c", p=128))
        w1 = cons.tile([128, 3, 604], BF16, tag="w1")
        nc.sync.dma_start(w1[:], d["w1"])
        fusw = cons.tile([128, 6, G], BF16, tag="fusw")
        nc.sync.dma_start(fusw[:], d["fusw"])
        fusb = cons.tile([1, G], BF16, tag="fusb")
        nc.sync.dma_start(fusb[:], d["fusb"])
        i128f = cons.tile([128, 128], F32R, tag="i128f")
        nc.sync.dma_start(i128f[:], d["i128f"])
        i128b = cons.tile([128, 128], BF16, tag="i128b")
        nc.sync.dma_start(i128b[:], d["i128b"])
        i128ff = cons.tile([128, 128], F32, tag="i128ff")
        nc.sync.dma_start(i128ff[:], d["i128f"].bitcast(F32))
        onescol = cons.tile([128, 1], BF16, tag="onescol")
        nc.vector.memset(onescol[:], 1.0)
        onesrow_bf = cons.tile([1, N], BF16, tag="onesrow_bf")
        nc.vector.memset(onesrow_bf[:], 1.0)
        onesrow_f = cons.tile([1, 128], F32, tag="onesrow_f")
        nc.vector.memset(onesrow_f[:], 1.0)
        if not pos_per_b:
            pos0 = cons.tile([128, 2, 1204], F32R, tag="pos0")
            nc.sync.dma_start(pos0[:], d["pos0"][0].rearrange("(m p) c -> p m c", p=128))
            pos1 = cons.tile([128, 2, 604], BF16, tag="pos1")
            nc.sync.dma_start(pos1[:], d["pos1"][0].rearrange("(m p) c -> p m c", p=128))
        if has_ln:
            lng = cons.tile([128, 4, G], F32, tag="lng")
            nc.sync.dma_start(lng[:], d["lng"])
            lnb = cons.tile([128, 4, G], F32, tag="lnb")
            nc.sync.dma_start(lnb[:], d["lnb"])

        AF = mybir.ActivationFunctionType
        OP = mybir.AluOpType

        def attention(sl_idx, whsb, fsrow, s, fdsb, mask, seed, bst4, bag4, slot):
            """softmax-attention for one stream-layer, through bn stats.

            whsb: sbuf bf16 [128, 2, 300] (Wh for this stream)
            fsrow: sbuf f32 [1, 512] (fs rows, s-major)
            fdsb: sbuf f32 [128, 2, 4-ish] holding fd columns at channel 2s
            mask: sbuf bf16 [128, 2, 256] 0/1 mask (eT orientation)
            seed(im, hP): emits residual-seeding matmuls into hP
                 (start=True ... stop=False); h' accumulates on top.
            bst4/bag4: [128, 4, 6]/[128, 4, 2] shared LN stats tiles; this
                 stream writes lanes slot*2 + im.
            returns hP psum APs [im] (LN+relu consumed later by ln_tail).
            """
            lr = sb.tile([128, 2, 256], F32, tag="lr", name="lr", bufs=6)
            for jm in range(2):
                nc.scalar.activation(
                    lr[:, jm, :], fsrow[:, 256 * s : 256 * (s + 1)], AF.Prelu,
                    bias=fdsb[:, jm, 2 * s : 2 * s + 1], alpha=0.2,
                )
            num = sb.tile([128, 2, 256], BF16, tag="num", name="num", bufs=6)
            nc.scalar.activation(num[:], lr[:], AF.Exp)
            numm = sb.tile([128, 2, 256], BF16, tag="numm", name="numm", bufs=6)
            nc.vector.tensor_mul(numm[:], num[:], mask[:])

            sRt = ps.tile([1, 256], F32, tag="SM", bufs=2, name=f"sR{sl_idx}")
            for jm in range(2):
                nc.tensor.matmul(
                    sRt[:], onescol[:], numm[:, jm, :], start=(jm == 0), stop=(jm == 1)
                )
            rr = sb.tile([1, 256], F32, tag="rr", name="rr", bufs=6)
            nc.vector.reciprocal_approx_fast(rr[:], sRt[:])
            recb = sb.tile([128, 256], F32, tag="recb", name="recb", bufs=6)
            nc.gpsimd.partition_broadcast(recb[:], rr[:])
            num_m = sb.tile([128, 2, 256], BF16, tag="num_m", name="num_m", bufs=6)
            for jm in range(2):
                nc.vector.tensor_mul(num_m[:, jm, :], numm[:, jm, :], recb[:])

            hPs = []
            for im in range(2):
                hPt = ps.tile([128, G], F32, tag="hP", bufs=4, name=f"hP{sl_idx}_{im}")
                hP = hPt[:]
                hPs.append(hP)
                seed(im, hP)
                for jm in range(2):
                    nc.tensor.matmul(
                        hP,
                        num_m[:, jm, 128 * im : 128 * (im + 1)],
                        whsb[:, jm, 0:G],
                        start=False,
                        stop=(jm == 1),
                    )
                k = 2 * slot + im
                nc.vector.bn_stats(bst4[:, k, :], hP)
                nc.vector.bn_aggr(bag4[:, k, :], bst4[:, k, :])
            return hPs

        def quake_rsqrt(u, x, k):
            """x = 1/sqrt(u) via Quake seed + 2 Newton iterations. [128,k] f32."""
            MAGIC = 0x5F3759DF
            t0 = sb.tile([128, k], F32, tag="rsq_t0", name="rsq_t0")
            nc.vector.tensor_scalar(
                t0[:].bitcast(I32), u.bitcast(I32), 1, None, OP.arith_shift_right
            )
            nc.vector.tensor_scalar(
                x.bitcast(I32), t0[:].bitcast(I32), MAGIC, -1, OP.subtract, OP.mult
            )
            for _ in range(2):
                sq = sb.tile([128, k], F32, tag="rsq_sq", name="rsq_sq")
                nc.vector.tensor_mul(sq[:], x, x)
                t = sb.tile([128, k], F32, tag="rsq_t", name="rsq_t")
                nc.vector.scalar_tensor_tensor(t[:], sq[:], 0.5, u, OP.mult, OP.mult)
                nc.vector.tensor_scalar(t[:], t[:], -1.0, 1.5, OP.mult, OP.add)
                nc.vector.tensor_mul(x, x, t[:])

        def ln_tail(layer, hPs_by_slot, bag4, sl_idxs, out_tags):
            """LN+relu tail for one or more stream slots of a layer.

            hPs_by_slot: {slot: [hP_im0, hP_im1]}; bag4 [128, 2*nslots, 2].
            Returns {slot: y sbuf bf16 [128, 2, 300]}.
            """
            slots = sorted(hPs_by_slot.keys())
            nk = 2 * len(slots)
            tsuf = f"{layer}_{sl_idxs[0]}"
            u = sb.tile([128, nk], F32, tag=f"u{tsuf}", name="u")
            nc.vector.tensor_scalar(u[:], bag4[:, :, 1], LN_EPS, None, OP.add)
            rstd = sb.tile([128, nk], F32, tag=f"rstd{tsuf}", name="rstd")
            quake_rsqrt(u[:], rstd[:], nk)
            nmr = sb.tile([128, nk], F32, tag=f"nmr{tsuf}", name="nmr")
            nc.vector.scalar_tensor_tensor(
                nmr[:], bag4[:, :, 0], -1.0, rstd[:], OP.mult, OP.mult
            )
            ys = {}
            for slot in slots:
                sl_idx = sl_idxs[slot]
                y = sb.tile([128, 2, G], BF16, tag=out_tags[slot], name=out_tags[slot])
                ys[slot] = y
                for im in range(2):
                    k = 2 * slot + im
                    if has_ln:
                        xn = sb.tile([128, G], F32, tag="xn", name="xn")
                        nc.scalar.activation(
                            xn[:], hPs_by_slot[slot][im], AF.Identity,
                            bias=nmr[:, k : k + 1], scale=rstd[:, k : k + 1],
                        )
                        xg = sb.tile([128, G], F32, tag="xg", name="xg")
                        nc.vector.scalar_tensor_tensor(
                            xg[:], xn[:], 1.0, lng[:, sl_idx, :], OP.mult, OP.mult
                        )
                        nc.vector.tensor_add(xg[:], xg[:], lnb[:, sl_idx, :])
                        nc.vector.tensor_scalar(y[:, im, :], xg[:], 0.0, None, OP.max)
                    else:
                        nc.scalar.activation(
                            y[:, im, :], hPs_by_slot[slot][im], AF.Relu,
                            bias=nmr[:, k : k + 1], scale=rstd[:, k : k + 1],
                        )
            return ys

        def transpose_y(y, tag):
            """y sbuf bf16 [128,2,300] -> yT sbuf bf16 [128,3,256] (K chunks)."""
            yT = sb.tile([128, 3, N], BF16, tag=tag, name=tag)
            for ci, (c0, cw) in enumerate(GCH):
                yTp = ps.tile([128, N], BF16, tag="PG", bufs=2, name="yTp")
                for im in range(2):
                    nc.tensor.transpose(
                        yTp[0:cw, 128 * im : 128 * (im + 1)],
                        y[:, im, c0 : c0 + cw],
                        i128b[:],
                    )
                if ci % 2 == 0:
                    nc.vector.tensor_copy(yT[0:cw, ci, :], yTp[0:cw, :])
                else:
                    nc.scalar.copy(yT[0:cw, ci, :], yTp[0:cw, :])
            return yT

        def fs_transposes(pe, name, ptag):
            """pe: sbuf f32 [128, 2, 4] (cols: synfd, synfs, semfd, semfs).
            Returns sbuf bf16 [128, 2, 256]: fs rows broadcast across
            partitions (dim 1 = stream)."""
            fsrowP = ps.tile([1, 512], F32, tag=ptag, name=f"fsP_{name}")
            for s in range(2):
                for m in range(2):
                    o = 256 * s + 128 * m
                    nc.tensor.transpose(
                        fsrowP[0:1, o : o + 128],
                        pe[:, m, 2 * s + 1 : 2 * s + 2],
                        i128ff[:],
                    )
            fsrow = sb.tile([1, 512], BF16, tag=f"fsrow_{name}", name=f"fsrow_{name}")
            nc.vector.tensor_copy(fsrow[:], fsrowP[:])
            fs_bc = sb.tile([128, 512], BF16, tag=f"fsbc_{name}", name=f"fsbc_{name}")
            nc.gpsimd.partition_broadcast(fs_bc[:], fsrow[:])
            return fs_bc

        # ================= per batch element =================
        # Software-pipelined: stage A(b) = input DMAs + L0 GEMM + fs rows
        # (depends only on inputs); stage B(b) = everything downstream.
        # A(b+1) is emitted before B(b) so the scheduler has independent
        # PE work during B's serial softmax/LN/transpose chains.
        def stage_a(b):
            pb = b if pos_per_b else 0
            if pos_per_b:
                pos0l = sb.tile([128, 2, 1204], F32R, tag="pos0b", bufs=4)
                nc.sync.dma_start(
                    pos0l[:], d["pos0"][pb].rearrange("(m p) c -> p m c", p=128)
                )
                pos1l = sb.tile([128, 2, 604], BF16, tag="pos1b", bufs=4)
                nc.sync.dma_start(
                    pos1l[:], d["pos1"][pb].rearrange("(m p) c -> p m c", p=128)
                )
            else:
                pos0l, pos1l = pos0, pos1

            hT = sb.tile([128, KC0, N], BF16, tag="hT", name="hT", bufs=4)
            nc.sync.dma_start(hT[:], d["hT"][b].rearrange("(k p) n -> p k n", p=128))
            m01_syn = sb.tile([128, 2, N], BF16, tag="m01_syn", name="m01_syn", bufs=4)
            nc.sync.dma_start(
                m01_syn[:], d["negms"][b].rearrange("(m p) n -> p m n", p=128)
            )
            m01_sem = sb.tile([128, 2, N], BF16, tag="m01_sem", name="m01_sem", bufs=4)
            nc.sync.dma_start(
                m01_sem[:], d["negmm"][b].rearrange("(m p) n -> p m n", p=128)
            )

            # ---- layer 0: both streams' Wh / fs/fd scores in one pass ----
            # w0 cols: [synW 0:300 | semW 300:600 | syn_tW 600:900 | sem_tW 900:1200
            #           | synfd, synfs, semfd, semfs 1200:1204]
            whsb0 = {}
            pe_sb = sb.tile([128, 2, 4], F32, tag="pe_sb", name="pe_sb", bufs=4)
            for s in range(2):
                whsb0[s] = sb.tile(
                    [128, 2, G], BF16, tag=f"whsb0_{s}", name=f"whsb0_{s}", bufs=4
                )
            for m in range(2):
                for s in range(2):
                    c0 = 302 * s
                    P0 = ps.tile([128, 302], F32, tag="PG", bufs=2, name="P0")
                    for k in range(KC0):
                        nc.tensor.matmul(
                            P0[:],
                            hT[:, k, 128 * m : 128 * (m + 1)],
                            w0[:, k, c0 : c0 + 302],
                            start=(k == 0),
                            stop=False,
                        )
                    nc.tensor.matmul(
                        P0[:],
                        i128f[:],
                        pos0l[:, m, c0 : c0 + 302],
                        start=False,
                        stop=True,
                    )
                    nc.scalar.copy(whsb0[s][:, m, :], P0[:, 0:300])
                    nc.scalar.copy(pe_sb[:, m, 2 * s : 2 * s + 2], P0[:, 300:302])

            fsrow0 = fs_transposes(pe_sb, "l0", "SM")
            return dict(
                hT=hT, m01_syn=m01_syn, m01_sem=m01_sem, whsb0=whsb0,
                pe_sb=pe_sb, fsrow0=fsrow0, pos0l=pos0l, pos1l=pos1l,
            )

        def stage_b(b, cx):
            hT = cx["hT"]
            m01_syn, m01_sem = cx["m01_syn"], cx["m01_sem"]
            whsb0, pe_sb, fsrow0 = cx["whsb0"], cx["pe_sb"], cx["fsrow0"]
            pos0l, pos1l = cx["pos0l"], cx["pos1l"]

            def seed_l0(s):
                def seed(im, hP):
                    c0 = 604 + s * G
                    for k in range(KC0):
                        nc.tensor.matmul(
                            hP,
                            hT[:, k, 128 * im : 128 * (im + 1)],
                            w0[:, k, c0 : c0 + G],
                            start=(k == 0),
                            stop=False,
                        )
                    if has_tb:
                        nc.tensor.matmul(
                            hP, i128f[:], pos0l[:, im, c0 : c0 + G],
                            start=False, stop=False,
                        )
                return seed

            ys = {}
            for s, mask in ((0, m01_syn), (1, m01_sem)):
                bst0 = sb.tile([128, 2, 6], F32, tag=f"bst0_{s}", name=f"bst0_{s}")
                bag0 = sb.tile([128, 2, 2], F32, tag=f"bag0_{s}", name=f"bag0_{s}")
                hp = attention(
                    s, whsb0[s], fsrow0, s, pe_sb, mask, seed_l0(s), bst0, bag0, 0
                )
                ys[s] = ln_tail(0, {0: hp}, bag0, (s,), (f"y{s}",))[0]

            # ---- layer 1 per stream ----
            # w1 cols: [synW1 0:300 | semW1 300:600 | synfd,synfs,semfd,semfs 600:604]
            y1 = {}
            yT1 = {}
            pe_sb1 = sb.tile([128, 2, 4], F32, tag="pe_sb1", name="pe_sb1")
            whsb1 = {}
            yTs = {}
            for s in range(2):
                yTs[s] = transpose_y(ys[s], f"yT0_{s}")
            for s in range(2):
                yT = yTs[s]
                whsb1[s] = sb.tile([128, 2, G], BF16, tag=f"whsb1_{s}", name=f"whsb1_{s}")
                for m in range(2):
                    c0 = 302 * s
                    P1 = ps.tile([128, 302], F32, tag="PG", bufs=2, name="P1")
                    for ki, (k0, kw) in enumerate(GCH):
                        nc.tensor.matmul(
                            P1[:],
                            yT[0:kw, ki, 128 * m : 128 * (m + 1)],
                            w1[0:kw, ki, c0 : c0 + 302],
                            start=(ki == 0),
                            stop=False,
                        )
                    nc.tensor.matmul(
                        P1[:],
                        i128b[:],
                        pos1l[:, m, c0 : c0 + 302],
                        start=False,
                        stop=True,
                    )
                    nc.vector.tensor_copy(whsb1[s][:, m, :], P1[:, 0:300])
                    nc.scalar.copy(pe_sb1[:, m, 2 * s : 2 * s + 2], P1[:, 300:302])
            fsrow1 = fs_transposes(pe_sb1, "l1", "SM")
            y1 = {}
            for s, mask in ((0, m01_syn), (1, m01_sem)):
                def seed_l1(im, hP, s=s):
                    nc.tensor.matmul(
                        hP, i128b[:], ys[s][:, im, :], start=True, stop=False
                    )
                bst1 = sb.tile([128, 2, 6], F32, tag=f"bst1_{s}", name=f"bst1_{s}")
                bag1 = sb.tile([128, 2, 2], F32, tag=f"bag1_{s}", name=f"bag1_{s}")
                hp = attention(
                    2 + s, whsb1[s], fsrow1, s, pe_sb1, mask, seed_l1, bst1, bag1, 0
                )
                y1[s] = ln_tail(1, {0: hp}, bag1, (2 + s,), (f"y{2+s}",))[0]
                yT1[s] = transpose_y(y1[s], f"yT1_{s}")

            # ---- fusion ----
            outsb = sb.tile([128, 2, G], F32, tag="outsb", name="outsb")
            for m in range(2):
                fP = ps.tile([128, G], F32, tag="PG", bufs=2, name="fP")
                first = True
                for s in range(2):
                    for ki, (k0, kw) in enumerate(GCH):
                        last = (s == 1 and ki == 2 and not has_fusb)
                        nc.tensor.matmul(
                            fP[:],
                            yT1[s][0:kw, ki, 128 * m : 128 * (m + 1)],
                            fusw[0:kw, 3 * s + ki, :],
                            start=first,
                            stop=last,
                        )
                        first = False
                if has_fusb:
                    nc.tensor.matmul(
                        fP[:],
                        onesrow_bf[0:1, 0:128],
                        fusb[:],
                        start=False,
                        stop=True,
                    )
                nc.scalar.activation(outsb[:, m, :], fP[:], AF.Relu)
            nc.sync.dma_start(
                out_d[b].rearrange("(m p) c -> p m c", p=128), outsb[:]
            )

        loop_ctx = tc.For_i(0, repeat, 1) if repeat > 1 else None
        if loop_ctx is not None:
            loop_ctx.__enter__()
        cx_next = stage_a(0)
        for b in range(n_b):
            cx = cx_next
            if b + 1 < n_b:
                cx_next = stage_a(b + 1)
            stage_b(b, cx)
        if loop_ctx is not None:
            loop_ctx.__exit__(None, None, None)

    nc.compile()
    return nc


def _host_pack(inputs):
    """Build all host-side arrays. Returns (per-core list of dicts, flags)."""
    h = np.asarray(inputs["h"], np.float32)
    adj = np.asarray(inputs["syntactic_adj"], np.float32)
    positions = np.asarray(inputs["positions"])

    hT = np.ascontiguousarray(h.transpose(0, 2, 1))
    # semantic graph mask on host (exact fp32, matches jax top_k tie-breaking)
    nrm = np.linalg.norm(h, axis=2, keepdims=True)
    hn = h / np.maximum(nrm, 1e-12)
    sim = np.matmul(hn, hn.transpose(0, 2, 1))  # [B,N,N] fp32
    order = np.argsort(-sim, axis=2, kind="stable")[:, :, :TOPK]
    maskA = np.zeros((h.shape[0], N, N), np.bool_)
    np.put_along_axis(maskA, order, True, axis=2)
    masksym = maskA | maskA.transpose(0, 2, 1)
    masksym |= np.eye(N, dtype=np.bool_)[None]  # reference adds +I unconditionally
    # 0/1 multiplicative masks, eT orientation (mask[j, i] = adj[i, j] > 0).
    # masksym is symmetric so no transpose needed for the semantic stream.
    negmm = masksym.astype(BF)
    negms = (adj.transpose(0, 2, 1) > 0).astype(BF)

    pos_same = bool((positions == positions[0:1]).all())
    pidx = positions[0] if pos_same else positions  # [N] or [B,N]

    def pack0(s):
        W = np.asarray(inputs[f"{s}0_W"], np.float64)
        asrc = np.asarray(inputs[f"{s}0_asrc"], np.float64)
        adst = np.asarray(inputs[f"{s}0_adst"], np.float64)
        return W, W @ adst, W @ asrc

    w0 = np.zeros((H, 1204), np.float64)
    pos_tabs0 = {}
    for si, s in enumerate(("syn", "sem")):
        W, wfd, wfs = pack0(s)
        w0[:, 302 * si : 302 * si + G] = W
        w0[:, 604 + si * G : 604 + (si + 1) * G] = np.asarray(inputs[f"{s}0_tW"], np.float64)
        w0[:, 302 * si + 300] = wfd
        w0[:, 302 * si + 301] = wfs
        pt = np.asarray(inputs[f"{s}0_pos"], np.float64)
        asrc = np.asarray(inputs[f"{s}0_asrc"], np.float64)
        adst = np.asarray(inputs[f"{s}0_adst"], np.float64)
        pos_tabs0[s] = (pt, pt @ adst, pt @ asrc)

    tb_syn = np.asarray(inputs["syn0_tb"], np.float64)
    tb_sem = np.asarray(inputs["sem0_tb"], np.float64)
    has_tb = bool(np.abs(tb_syn).max() > 0 or np.abs(tb_sem).max() > 0)

    def build_pos0(pidx1):  # pidx1: [N] int
        p = np.zeros((N, 1204), np.float64)
        for si, s in enumerate(("syn", "sem")):
            pt, pfd, pfs = pos_tabs0[s]
            p[:, 302 * si : 302 * si + G] = pt[pidx1]
            p[:, 302 * si + 300] = pfd[pidx1]
            p[:, 302 * si + 301] = pfs[pidx1]
        if has_tb:
            p[:, 604:904] = tb_syn[None, :]
            p[:, 904:1204] = tb_sem[None, :]
        return p

    w1 = np.zeros((384, 604), np.float64)
    pos_tabs1 = {}
    for si, s in enumerate(("syn", "sem")):
        W = np.asarray(inputs[f"{s}1_W"], np.float64)
        asrc = np.asarray(inputs[f"{s}1_asrc"], np.float64)
        adst = np.asarray(inputs[f"{s}1_adst"], np.float64)
        w1[:G, 302 * si : 302 * si + G] = W
        w1[:G, 302 * si + 300] = W @ adst
        w1[:G, 302 * si + 301] = W @ asrc
        pt = np.asarray(inputs[f"{s}1_pos"], np.float64)
        pos_tabs1[s] = (pt, pt @ adst, pt @ asrc)

    def build_pos1(pidx1):
        p = np.zeros((N, 604), np.float64)
        for si, s in enumerate(("syn", "sem")):
            pt, pfd, pfs = pos_tabs1[s]
            p[:, 302 * si : 302 * si + G] = pt[pidx1]
            p[:, 302 * si + 300] = pfd[pidx1]
            p[:, 302 * si + 301] = pfs[pidx1]
        return p

    # w1 pre-chunked to [128, 3, 604]
    w1c = np.zeros((128, 3, 604), np.float64)
    for ki, (k0, kw) in enumerate(GCH):
        w1c[:kw, ki, :] = w1[k0 : k0 + kw, :]

    fw = np.asarray(inputs["fus_W"], np.float64)  # [600, 300]
    fusw = np.zeros((128, 6, G), np.float64)
    for s in range(2):
        for ki, (k0, kw) in enumerate(GCH):
            fusw[:kw, 3 * s + ki, :] = fw[300 * s + k0 : 300 * s + k0 + kw, :]
    fusb = np.asarray(inputs["fus_b"], np.float64)[None, :]
    has_fusb = bool(np.abs(fusb).max() > 0)

    lngs = [np.asarray(inputs[k], np.float32) for k in ("syn0_lng", "sem0_lng", "syn1_lng", "sem1_lng")]
    lnbs = [np.asarray(inputs[k], np.float32) for k in ("syn0_lnb", "sem0_lnb", "syn1_lnb", "sem1_lnb")]
    has_ln = bool(
        any(np.abs(g - 1.0).max() > 0 for g in lngs) or any(np.abs(bb).max() > 0 for bb in lnbs)
    )

    shared = {
        "w0": w0.astype(BF),
        "w1": w1c.astype(BF),
        "fusw": fusw.astype(BF),
        "fusb": fusb.astype(BF),
        "i128f": np.eye(128, dtype=np.float32),
        "i128b": np.eye(128).astype(BF),
    }
    if has_ln:
        shared["lng"] = np.stack(
            [np.broadcast_to(g, (128, G)) for g in lngs], axis=1
        ).astype(np.float32).copy()
        shared["lnb"] = np.stack(
            [np.broadcast_to(bb, (128, G)) for bb in lnbs], axis=1
        ).astype(np.float32).copy()

    if pos_same:
        shared["pos0"] = build_pos0(pidx)[None].astype(np.float32)
        shared["pos1"] = build_pos1(pidx)[None].astype(BF)
        pos_per_b = False
    else:
        pos_per_b = True

    in_maps = []
    for c in range(NCORES):
        sl = slice(c * BL, (c + 1) * BL)
        m = dict(shared)
        m["hT"] = hT[sl].astype(BF)
        m["negms"] = negms[sl]
        m["negmm"] = negmm[sl]
        if pos_per_b:
            m["pos0"] = np.stack([build_pos0(positions[i]) for i in range(c * BL, (c + 1) * BL)]).astype(np.float32)
            m["pos1"] = np.stack([build_pos1(positions[i]) for i in range(c * BL, (c + 1) * BL)]).astype(BF)
        in_maps.append(m)

    flags = (BL, pos_per_b, has_tb, has_ln, has_fusb)
    return in_maps, flags


def _get_program(flags):
    if flags not in _prog_cache:
        _prog_cache[flags] = _build_program(*flags)
    return _prog_cache[flags]


_last_results = {}


def kernel(**inputs):
    in_maps, flags = _host_pack(inputs)
    nc = _get_program(flags)
    res = run_bass_kernel_spmd(nc, in_maps, list(range(NCORES)))
    _last_results["res"] = res
    out = np.concatenate([res.results[c]["out"] for c in range(NCORES)], axis=0)
    return np.ascontiguousarray(out.astype(np.float32))


# revision 30
# speedup vs baseline: 1.2696x; 1.0235x over previous
"""Trainium2 Bass kernel for the dual-stream position-aware GAT (EAGLE_V2).

Data-parallel over batch B=128 across 8 NeuronCores (16 batch elems/core).
Host pre-transposes h, pre-packs weights, and builds the 0/1 attention
masks (incl. the semantic top-K graph). The device program per batch
element runs 2 GAT layers per stream (syn/sem) with a fused
softmax-attention + LayerNorm+ReLU, then the fusion projection.

v2 rewrite vs baseline: eT scores built via one rank-1 matmul + ACT
Prelu-with-bias (fd as per-partition bias), multiplicative 0/1 mask on
DVE, softmax denominator/broadcast via tiny PE matmuls, LN rstd via ACT
Ln/Exp (one act table), zero GpSimd work, and PSUM split into dedicated
bank rings (Pmm/attA/hP) so batch elements pipeline.

Self-contained: hardcodes all shapes from the problem spec.
"""
import os
import sys

sys.path.insert(0, "/opt/trn_rl_repo")
os.environ.setdefault("MYCRO_LOCAL_CACHE", "1")

from contextlib import ExitStack

import ml_dtypes
import numpy as np

import concourse.bass as bass
import concourse.tile as tile
from concourse import bacc, mybir
from concourse.bass_utils import run_bass_kernel_spmd

B, N, H, G, TOPK = 128, 256, 768, 300, 10
NCORES = 8
BL = B // NCORES
LN_EPS = 1e-5
F32 = mybir.dt.float32
F32R = mybir.dt.float32r
I32 = mybir.dt.int32
BF16 = mybir.dt.bfloat16
BF = ml_dtypes.bfloat16

KC0 = H // 128  # 6 K-chunks for the H contraction
# L1 / fusion contraction chunks over G=300: 128, 128, 44
GCH = [(0, 128), (128, 128), (256, 44)]

_prog_cache = {}


def _build_program(n_b, pos_per_b, has_tb, has_ln, has_fusb, repeat=1):
    nc = bacc.Bacc("TRN2", target_bir_lowering=False, debug=False)

    d = {}
    d["hT"] = nc.dram_tensor("hT", [n_b, H, N], BF16, kind="ExternalInput").ap()
    # 0/1 masks in eT orientation: mask[j, i] = adj[i, j] > 0
    d["negms"] = nc.dram_tensor("negms", [n_b, N, N], BF16, kind="ExternalInput").ap()
    d["negmm"] = nc.dram_tensor("negmm", [n_b, N, N], BF16, kind="ExternalInput").ap()
    d["w0"] = nc.dram_tensor("w0", [H, 1204], BF16, kind="ExternalInput").ap()
    np0 = n_b if pos_per_b else 1
    d["pos0"] = nc.dram_tensor("pos0", [np0, N, 1204], F32R, kind="ExternalInput").ap()
    d["w1"] = nc.dram_tensor("w1", [128, 3, 604], BF16, kind="ExternalInput").ap()
    d["pos1"] = nc.dram_tensor("pos1", [np0, N, 604], BF16, kind="ExternalInput").ap()
    d["fusw"] = nc.dram_tensor("fusw", [128, 6, G], BF16, kind="ExternalInput").ap()
    d["fusb"] = nc.dram_tensor("fusb", [1, G], BF16, kind="ExternalInput").ap()
    d["i128f"] = nc.dram_tensor("i128f", [128, 128], F32R, kind="ExternalInput").ap()
    d["i128b"] = nc.dram_tensor("i128b", [128, 128], BF16, kind="ExternalInput").ap()
    if has_ln:
        d["lng"] = nc.dram_tensor("lng", [128, 4, G], F32, kind="ExternalInput").ap()
        d["lnb"] = nc.dram_tensor("lnb", [128, 4, G], F32, kind="ExternalInput").ap()
    out_d = nc.dram_tensor("out", [n_b, N, G], F32, kind="ExternalOutput").ap()

    with tile.TileContext(nc) as tc, ExitStack() as ctx:
        cons = ctx.enter_context(tc.tile_pool(name="cons", bufs=1))
        sb = ctx.enter_context(tc.tile_pool(name="sb", bufs=3))
        ps = ctx.enter_context(tc.tile_pool(name="ps", bufs=2, space="PSUM"))

        # ---- constants / weights (loaded once) ----
        w0 = cons.tile([128, KC0, 1204], BF16, tag="w0")
        nc.sync.dma_start(w0[:], d["w0"].rearrange("(k p) c -> p k c", p=128))
        w1 = cons.tile([128, 3, 604], BF16, tag="w1")
        nc.sync.dma_start(w1[:], d["w1"])
        fusw = cons.tile([128, 6, G], BF16, tag="fusw")
        nc.sync.dma_start(fusw[:], d["fusw"])
        fusb = cons.tile([1, G], BF16, tag="fusb")
        nc.sync.dma_start(fusb[:], d["fusb"])
        i128f = cons.tile([128, 128], F32R, tag="i128f")
        nc.sync.dma_start(i128f[:], d["i128f"])
        i128b = cons.tile([128, 128], BF16, tag="i128b")
        nc.sync.dma_start(i128b[:], d["i128b"])
        i128ff = cons.tile([128, 128], F32, tag="i128ff")
        nc.sync.dma_start(i128ff[:], d["i128f"].bitcast(F32))
        onescol = cons.tile([128, 1], BF16, tag="onescol")
        nc.vector.memset(onescol[:], 1.0)
        onesrow_bf = cons.tile([1, N], BF16, tag="onesrow_bf")
        nc.vector.memset(onesrow_bf[:], 1.0)
        onesrow_f = cons.tile([1, 128], F32, tag="onesrow_f")
        nc.vector.memset(onesrow_f[:], 1.0)
        if not pos_per_b:
            pos0 = cons.tile([128, 2, 1204], F32R, tag="pos0")
            nc.sync.dma_start(pos0[:], d["pos0"][0].rearrange("(m p) c -> p m c", p=128))
            pos1 = cons.tile([128, 2, 604], BF16, tag="pos1")
            nc.sync.dma_start(pos1[:], d["pos1"][0].rearrange("(m p) c -> p m c", p=128))
        if has_ln:
            lng = cons.tile([128, 4, G], F32, tag="lng")
            nc.sync.dma_start(lng[:], d["lng"])
            lnb = cons.tile([128, 4, G], F32, tag="lnb")
            nc.sync.dma_start(lnb[:], d["lnb"])

        AF = mybir.ActivationFunctionType
        OP = mybir.AluOpType

        def attention(sl_idx, whsb, fsrow, s, fdsb, mask, seed, bst4, bag4, slot):
            """softmax-attention for one stream-layer, through bn stats.

            whsb: sbuf bf16 [128, 2, 300] (Wh for this stream)
            fsrow: sbuf f32 [1, 512] (fs rows, s-major)
            fdsb: sbuf f32 [128, 2, 4-ish] holding fd columns at channel 2s
            mask: sbuf bf16 [128, 2, 256] 0/1 mask (eT orientation)
            seed(im, hP): emits residual-seeding matmuls into hP
                 (start=True ... stop=False); h' accumulates on top.
            bst4/bag4: [128, 4, 6]/[128, 4, 2] shared LN stats tiles; this
                 stream writes lanes slot*2 + im.
            returns hP psum APs [im] (LN+relu consumed later by ln_tail).
            """
            lr = sb.tile([128, 2, 256], F32, tag="lr", name="lr", bufs=6)
            for jm in range(2):
                nc.scalar.activation(
                    lr[:, jm, :], fsrow[:, 256 * s : 256 * (s + 1)], AF.Prelu,
                    bias=fdsb[:, jm, 2 * s : 2 * s + 1], alpha=0.2,
                )
            num = sb.tile([128, 2, 256], BF16, tag="num", name="num", bufs=6)
            nc.scalar.activation(num[:], lr[:], AF.Exp)
            numm = sb.tile([128, 2, 256], BF16, tag="numm", name="numm", bufs=6)
            nc.vector.tensor_mul(numm[:], num[:], mask[:])

            sRt = ps.tile([1, 256], F32, tag="SM", bufs=2, name=f"sR{sl_idx}")
            for jm in range(2):
                nc.tensor.matmul(
                    sRt[:], onescol[:], numm[:, jm, :], start=(jm == 0), stop=(jm == 1)
                )
            rr = sb.tile([1, 256], F32, tag="rr", name="rr", bufs=6)
            nc.vector.reciprocal_approx_fast(rr[:], sRt[:])
            recb = sb.tile([128, 256], F32, tag="recb", name="recb", bufs=6)
            nc.gpsimd.partition_broadcast(recb[:], rr[:])
            num_m = sb.tile([128, 2, 256], BF16, tag="num_m", name="num_m", bufs=6)
            for jm in range(2):
                nc.vector.tensor_mul(num_m[:, jm, :], numm[:, jm, :], recb[:])

            hPs = []
            for im in range(2):
                hPt = ps.tile([128, G], F32, tag="hP", bufs=4, name=f"hP{sl_idx}_{im}")
                hP = hPt[:]
                hPs.append(hP)
                seed(im, hP)
                for jm in range(2):
                    nc.tensor.matmul(
                        hP,
                        num_m[:, jm, 128 * im : 128 * (im + 1)],
                        whsb[:, jm, 0:G],
                        start=False,
                        stop=(jm == 1),
                    )
                k = 2 * slot + im
                nc.vector.bn_stats(bst4[:, k, :], hP)
                nc.vector.bn_aggr(bag4[:, k, :], bst4[:, k, :])
            return hPs

        def quake_rsqrt(u, x, k):
            """x = 1/sqrt(u) via Quake seed + 2 Newton iterations. [128,k] f32."""
            MAGIC = 0x5F3759DF
            t0 = sb.tile([128, k], F32, tag="rsq_t0", name="rsq_t0")
            nc.vector.tensor_scalar(
                t0[:].bitcast(I32), u.bitcast(I32), 1, None, OP.arith_shift_right
            )
            nc.vector.tensor_scalar(
                x.bitcast(I32), t0[:].bitcast(I32), MAGIC, -1, OP.subtract, OP.mult
            )
            for _ in range(1):
                sq = sb.tile([128, k], F32, tag="rsq_sq", name="rsq_sq")
                nc.vector.tensor_mul(sq[:], x, x)
                t = sb.tile([128, k], F32, tag="rsq_t", name="rsq_t")
                nc.vector.scalar_tensor_tensor(t[:], sq[:], 0.5, u, OP.mult, OP.mult)
                nc.vector.tensor_scalar(t[:], t[:], -1.0, 1.5, OP.mult, OP.add)
                nc.vector.tensor_mul(x, x, t[:])

        def ln_tail(layer, hPs_by_slot, bag4, sl_idxs, out_tags):
            """LN+relu tail for one or more stream slots of a layer.

            hPs_by_slot: {slot: [hP_im0, hP_im1]}; bag4 [128, 2*nslots, 2].
            Returns {slot: y sbuf bf16 [128, 2, 300]}.
            """
            slots = sorted(hPs_by_slot.keys())
            nk = 2 * len(slots)
            tsuf = f"{layer}_{sl_idxs[0]}"
            u = sb.tile([128, nk], F32, tag=f"u{tsuf}", name="u")
            nc.vector.tensor_scalar(u[:], bag4[:, :, 1], LN_EPS, None, OP.add)
            rstd = sb.tile([128, nk], F32, tag=f"rstd{tsuf}", name="rstd")
            quake_rsqrt(u[:], rstd[:], nk)
            nmr = sb.tile([128, nk], F32, tag=f"nmr{tsuf}", name="nmr")
            nc.vector.scalar_tensor_tensor(
                nmr[:], bag4[:, :, 0], -1.0, rstd[:], OP.mult, OP.mult
            )
            ys = {}
            for slot in slots:
                sl_idx = sl_idxs[slot]
                y = sb.tile([128, 2, G], BF16, tag=out_tags[slot], name=out_tags[slot])
                ys[slot] = y
                for im in range(2):
                    k = 2 * slot + im
                    if has_ln:
                        xn = sb.tile([128, G], F32, tag="xn", name="xn")
                        nc.scalar.activation(
                            xn[:], hPs_by_slot[slot][im], AF.Identity,
                            bias=nmr[:, k : k + 1], scale=rstd[:, k : k + 1],
                        )
                        xg = sb.tile([128, G], F32, tag="xg", name="xg")
                        nc.vector.scalar_tensor_tensor(
                            xg[:], xn[:], 1.0, lng[:, sl_idx, :], OP.mult, OP.mult
                        )
                        nc.vector.tensor_add(xg[:], xg[:], lnb[:, sl_idx, :])
                        nc.vector.tensor_scalar(y[:, im, :], xg[:], 0.0, None, OP.max)
                    else:
                        nc.scalar.activation(
                            y[:, im, :], hPs_by_slot[slot][im], AF.Relu,
                            bias=nmr[:, k : k + 1], scale=rstd[:, k : k + 1],
                        )
            return ys

        def transpose_y(y, tag):
            """y sbuf bf16 [128,2,300] -> yT sbuf bf16 [128,3,256] (K chunks)."""
            yT = sb.tile([128, 3, N], BF16, tag=tag, name=tag)
            for ci, (c0, cw) in enumerate(GCH):
                yTp = ps.tile([128, N], BF16, tag="PG", bufs=2, name="yTp")
                for im in range(2):
                    nc.tensor.transpose(
                        yTp[0:cw, 128 * im : 128 * (im + 1)],
                        y[:, im, c0 : c0 + cw],
                        i128b[:],
                    )
                if ci % 2 == 0:
                    nc.vector.tensor_copy(yT[0:cw, ci, :], yTp[0:cw, :])
                else:
                    nc.scalar.copy(yT[0:cw, ci, :], yTp[0:cw, :])
            return yT

        def fs_transposes(pe, name, ptag):
            """pe: sbuf f32 [128, 2, 4] (cols: synfd, synfs, semfd, semfs).
            Returns sbuf bf16 [128, 2, 256]: fs rows broadcast across
            partitions (dim 1 = stream)."""
            fsrowP = ps.tile([1, 512], F32, tag=ptag, name=f"fsP_{name}")
            for s in range(2):
                for m in range(2):
                    o = 256 * s + 128 * m
                    nc.tensor.transpose(
                        fsrowP[0:1, o : o + 128],
                        pe[:, m, 2 * s + 1 : 2 * s + 2],
                        i128ff[:],
                    )
            fsrow = sb.tile([1, 512], BF16, tag=f"fsrow_{name}", name=f"fsrow_{name}")
            nc.vector.tensor_copy(fsrow[:], fsrowP[:])
            fs_bc = sb.tile([128, 512], BF16, tag=f"fsbc_{name}", name=f"fsbc_{name}")
            nc.gpsimd.partition_broadcast(fs_bc[:], fsrow[:])
            return fs_bc

        # ================= per batch element =================
        # Software-pipelined: stage A(b) = input DMAs + L0 GEMM + fs rows
        # (depends only on inputs); stage B(b) = everything downstream.
        # A(b+1) is emitted before B(b) so the scheduler has independent
        # PE work during B's serial softmax/LN/transpose chains.
        def stage_a(b):
            pb = b if pos_per_b else 0
            if pos_per_b:
                pos0l = sb.tile([128, 2, 1204], F32R, tag="pos0b", bufs=4)
                nc.sync.dma_start(
                    pos0l[:], d["pos0"][pb].rearrange("(m p) c -> p m c", p=128)
                )
                pos1l = sb.tile([128, 2, 604], BF16, tag="pos1b", bufs=4)
                nc.sync.dma_start(
                    pos1l[:], d["pos1"][pb].rearrange("(m p) c -> p m c", p=128)
                )
            else:
                pos0l, pos1l = pos0, pos1

            hT = sb.tile([128, KC0, N], BF16, tag="hT", name="hT", bufs=4)
            nc.sync.dma_start(hT[:], d["hT"][b].rearrange("(k p) n -> p k n", p=128))
            m01_syn = sb.tile([128, 2, N], BF16, tag="m01_syn", name="m01_syn", bufs=4)
            nc.sync.dma_start(
                m01_syn[:], d["negms"][b].rearrange("(m p) n -> p m n", p=128)
            )
            m01_sem = sb.tile([128, 2, N], BF16, tag="m01_sem", name="m01_sem", bufs=4)
            nc.sync.dma_start(
                m01_sem[:], d["negmm"][b].rearrange("(m p) n -> p m n", p=128)
            )

            # ---- layer 0: both streams' Wh / fs/fd scores in one pass ----
            # w0 cols: [synW 0:300 | semW 300:600 | syn_tW 600:900 | sem_tW 900:1200
            #           | synfd, synfs, semfd, semfs 1200:1204]
            whsb0 = {}
            pe_sb = sb.tile([128, 2, 4], F32, tag="pe_sb", name="pe_sb", bufs=4)
            for s in range(2):
                whsb0[s] = sb.tile(
                    [128, 2, G], BF16, tag=f"whsb0_{s}", name=f"whsb0_{s}", bufs=4
                )
            for m in range(2):
                for s in range(2):
                    c0 = 302 * s
                    P0 = ps.tile([128, 302], F32, tag="PG", bufs=2, name="P0")
                    for k in range(KC0):
                        nc.tensor.matmul(
                            P0[:],
                            hT[:, k, 128 * m : 128 * (m + 1)],
                            w0[:, k, c0 : c0 + 302],
                            start=(k == 0),
                            stop=False,
                        )
                    nc.tensor.matmul(
                        P0[:],
                        i128f[:],
                        pos0l[:, m, c0 : c0 + 302],
                        start=False,
                        stop=True,
                    )
                    nc.scalar.copy(whsb0[s][:, m, :], P0[:, 0:300])
                    nc.scalar.copy(pe_sb[:, m, 2 * s : 2 * s + 2], P0[:, 300:302])

            fsrow0 = fs_transposes(pe_sb, "l0", "SM")
            return dict(
                hT=hT, m01_syn=m01_syn, m01_sem=m01_sem, whsb0=whsb0,
                pe_sb=pe_sb, fsrow0=fsrow0, pos0l=pos0l, pos1l=pos1l,
            )

        def stage_b(b, cx):
            hT = cx["hT"]
            m01_syn, m01_sem = cx["m01_syn"], cx["m01_sem"]
            whsb0, pe_sb, fsrow0 = cx["whsb0"], cx["pe_sb"], cx["fsrow0"]
            pos0l, pos1l = cx["pos0l"], cx["pos1l"]

            def seed_l0(s):
                def seed(im, hP):
                    c0 = 604 + s * G
                    for k in range(KC0):
                        nc.tensor.matmul(
                            hP,
                            hT[:, k, 128 * im : 128 * (im + 1)],
                            w0[:, k, c0 : c0 + G],
                            start=(k == 0),
                            stop=False,
                        )
                    if has_tb:
                        nc.tensor.matmul(
                            hP, i128f[:], pos0l[:, im, c0 : c0 + G],
                            start=False, stop=False,
                        )
                return seed

            ys = {}
            for s, mask in ((0, m01_syn), (1, m01_sem)):
                bst0 = sb.tile([128, 2, 6], F32, tag=f"bst0_{s}", name=f"bst0_{s}")
                bag0 = sb.tile([128, 2, 2], F32, tag=f"bag0_{s}", name=f"bag0_{s}")
                hp = attention(
                    s, whsb0[s], fsrow0, s, pe_sb, mask, seed_l0(s), bst0, bag0, 0
                )
                ys[s] = ln_tail(0, {0: hp}, bag0, (s,), (f"y{s}",))[0]

            # ---- layer 1 per stream ----
            # w1 cols: [synW1 0:300 | semW1 300:600 | synfd,synfs,semfd,semfs 600:604]
            y1 = {}
            yT1 = {}
            pe_sb1 = sb.tile([128, 2, 4], F32, tag="pe_sb1", name="pe_sb1")
            whsb1 = {}
            yTs = {}
            for s in range(2):
                yTs[s] = transpose_y(ys[s], f"yT0_{s}")
            for s in range(2):
                yT = yTs[s]
                whsb1[s] = sb.tile([128, 2, G], BF16, tag=f"whsb1_{s}", name=f"whsb1_{s}")
                for m in range(2):
                    c0 = 302 * s
                    P1 = ps.tile([128, 302], F32, tag="PG", bufs=2, name="P1")
                    for ki, (k0, kw) in enumerate(GCH):
                        nc.tensor.matmul(
                            P1[:],
                            yT[0:kw, ki, 128 * m : 128 * (m + 1)],
                            w1[0:kw, ki, c0 : c0 + 302],
                            start=(ki == 0),
                            stop=False,
                        )
                    nc.tensor.matmul(
                        P1[:],
                        i128b[:],
                        pos1l[:, m, c0 : c0 + 302],
                        start=False,
                        stop=True,
                    )
                    if m == 0:
                        nc.vector.tensor_copy(whsb1[s][:, m, :], P1[:, 0:300])
                    else:
                        nc.scalar.copy(whsb1[s][:, m, :], P1[:, 0:300])
                    nc.scalar.copy(pe_sb1[:, m, 2 * s : 2 * s + 2], P1[:, 300:302])
            fsrow1 = fs_transposes(pe_sb1, "l1", "SM")
            y1 = {}
            for s, mask in ((0, m01_syn), (1, m01_sem)):
                def seed_l1(im, hP, s=s):
                    nc.tensor.matmul(
                        hP, i128b[:], ys[s][:, im, :], start=True, stop=False
                    )
                bst1 = sb.tile([128, 2, 6], F32, tag=f"bst1_{s}", name=f"bst1_{s}")
                bag1 = sb.tile([128, 2, 2], F32, tag=f"bag1_{s}", name=f"bag1_{s}")
                hp = attention(
                    2 + s, whsb1[s], fsrow1, s, pe_sb1, mask, seed_l1, bst1, bag1, 0
                )
                y1[s] = ln_tail(1, {0: hp}, bag1, (2 + s,), (f"y{2+s}",))[0]

            # ---- fusion (fP slots interleaved with the yT1 transposes so
            # the syn half of each fusion group starts without waiting on
            # the sem stream's LN/transpose tail) ----
            outsb = sb.tile([128, 2, G], F32, tag="outsb", name="outsb")
            fPs = []
            yT1[0] = transpose_y(y1[0], "yT1_0")
            fPs.append(ps.tile([128, G], F32, tag="PG", bufs=2, name="fP0"))
            yT1[1] = transpose_y(y1[1], "yT1_1")
            fPs.append(ps.tile([128, G], F32, tag="PG", bufs=2, name="fP1"))
            for m in range(2):
                fP = fPs[m]
                first = True
                for s in range(2):
                    for ki, (k0, kw) in enumerate(GCH):
                        last = (s == 1 and ki == 2 and not has_fusb)
                        nc.tensor.matmul(
                            fP[:],
                            yT1[s][0:kw, ki, 128 * m : 128 * (m + 1)],
                            fusw[0:kw, 3 * s + ki, :],
                            start=first,
                            stop=last,
                        )
                        first = False
                if has_fusb:
                    nc.tensor.matmul(
                        fP[:],
                        onesrow_bf[0:1, 0:128],
                        fusb[:],
                        start=False,
                        stop=True,
                    )
                nc.scalar.activation(outsb[:, m, :], fP[:], AF.Relu)
            nc.sync.dma_start(
                out_d[b].rearrange("(m p) c -> p m c", p=128), outsb[:]
            )

        loop_ctx = tc.For_i(0, repeat, 1) if repeat > 1 else None
        if loop_ctx is not None:
            loop_ctx.__enter__()
        cx_next = stage_a(0)
        for b in range(n_b):
            cx = cx_next
            if b + 1 < n_b:
                cx_next = stage_a(b + 1)
            stage_b(b, cx)
        if loop_ctx is not None:
            loop_ctx.__exit__(None, None, None)

    nc.compile()
    return nc


def _host_pack(inputs):
    """Build all host-side arrays. Returns (per-core list of dicts, flags)."""
    h = np.asarray(inputs["h"], np.float32)
    adj = np.asarray(inputs["syntactic_adj"], np.float32)
    positions = np.asarray(inputs["positions"])

    hT = np.ascontiguousarray(h.transpose(0, 2, 1))
    # semantic graph mask on host (exact fp32, matches jax top_k tie-breaking)
    nrm = np.linalg.norm(h, axis=2, keepdims=True)
    hn = h / np.maximum(nrm, 1e-12)
    sim = np.matmul(hn, hn.transpose(0, 2, 1))  # [B,N,N] fp32
    order = np.argsort(-sim, axis=2, kind="stable")[:, :, :TOPK]
    maskA = np.zeros((h.shape[0], N, N), np.bool_)
    np.put_along_axis(maskA, order, True, axis=2)
    masksym = maskA | maskA.transpose(0, 2, 1)
    masksym |= np.eye(N, dtype=np.bool_)[None]  # reference adds +I unconditionally
    # 0/1 multiplicative masks, eT orientation (mask[j, i] = adj[i, j] > 0).
    # masksym is symmetric so no transpose needed for the semantic stream.
    negmm = masksym.astype(BF)
    negms = (adj.transpose(0, 2, 1) > 0).astype(BF)

    pos_same = bool((positions == positions[0:1]).all())
    pidx = positions[0] if pos_same else positions  # [N] or [B,N]

    def pack0(s):
        W = np.asarray(inputs[f"{s}0_W"], np.float64)
        asrc = np.asarray(inputs[f"{s}0_asrc"], np.float64)
        adst = np.asarray(inputs[f"{s}0_adst"], np.float64)
        return W, W @ adst, W @ asrc

    w0 = np.zeros((H, 1204), np.float64)
    pos_tabs0 = {}
    for si, s in enumerate(("syn", "sem")):
        W, wfd, wfs = pack0(s)
        w0[:, 302 * si : 302 * si + G] = W
        w0[:, 604 + si * G : 604 + (si + 1) * G] = np.asarray(inputs[f"{s}0_tW"], np.float64)
        w0[:, 302 * si + 300] = wfd
        w0[:, 302 * si + 301] = wfs
        pt = np.asarray(inputs[f"{s}0_pos"], np.float64)
        asrc = np.asarray(inputs[f"{s}0_asrc"], np.float64)
        adst = np.asarray(inputs[f"{s}0_adst"], np.float64)
        pos_tabs0[s] = (pt, pt @ adst, pt @ asrc)

    tb_syn = np.asarray(inputs["syn0_tb"], np.float64)
    tb_sem = np.asarray(inputs["sem0_tb"], np.float64)
    has_tb = bool(np.abs(tb_syn).max() > 0 or np.abs(tb_sem).max() > 0)

    def build_pos0(pidx1):  # pidx1: [N] int
        p = np.zeros((N, 1204), np.float64)
        for si, s in enumerate(("syn", "sem")):
            pt, pfd, pfs = pos_tabs0[s]
            p[:, 302 * si : 302 * si + G] = pt[pidx1]
            p[:, 302 * si + 300] = pfd[pidx1]
            p[:, 302 * si + 301] = pfs[pidx1]
        if has_tb:
            p[:, 604:904] = tb_syn[None, :]
            p[:, 904:1204] = tb_sem[None, :]
        return p

    w1 = np.zeros((384, 604), np.float64)
    pos_tabs1 = {}
    for si, s in enumerate(("syn", "sem")):
        W = np.asarray(inputs[f"{s}1_W"], np.float64)
        asrc = np.asarray(inputs[f"{s}1_asrc"], np.float64)
        adst = np.asarray(inputs[f"{s}1_adst"], np.float64)
        w1[:G, 302 * si : 302 * si + G] = W
        w1[:G, 302 * si + 300] = W @ adst
        w1[:G, 302 * si + 301] = W @ asrc
        pt = np.asarray(inputs[f"{s}1_pos"], np.float64)
        pos_tabs1[s] = (pt, pt @ adst, pt @ asrc)

    def build_pos1(pidx1):
        p = np.zeros((N, 604), np.float64)
        for si, s in enumerate(("syn", "sem")):
            pt, pfd, pfs = pos_tabs1[s]
            p[:, 302 * si : 302 * si + G] = pt[pidx1]
            p[:, 302 * si + 300] = pfd[pidx1]
            p[:, 302 * si + 301] = pfs[pidx1]
        return p

    # w1 pre-chunked to [128, 3, 604]
    w1c = np.zeros((128, 3, 604), np.float64)
    for ki, (k0, kw) in enumerate(GCH):
        w1c[:kw, ki, :] = w1[k0 : k0 + kw, :]

    fw = np.asarray(inputs["fus_W"], np.float64)  # [600, 300]
    fusw = np.zeros((128, 6, G), np.float64)
    for s in range(2):
        for ki, (k0, kw) in enumerate(GCH):
            fusw[:kw, 3 * s + ki, :] = fw[300 * s + k0 : 300 * s + k0 + kw, :]
    fusb = np.asarray(inputs["fus_b"], np.float64)[None, :]
    has_fusb = bool(np.abs(fusb).max() > 0)

    lngs = [np.asarray(inputs[k], np.float32) for k in ("syn0_lng", "sem0_lng", "syn1_lng", "sem1_lng")]
    lnbs = [np.asarray(inputs[k], np.float32) for k in ("syn0_lnb", "sem0_lnb", "syn1_lnb", "sem1_lnb")]
    has_ln = bool(
        any(np.abs(g - 1.0).max() > 0 for g in lngs) or any(np.abs(bb).max() > 0 for bb in lnbs)
    )

    shared = {
        "w0": w0.astype(BF),
        "w1": w1c.astype(BF),
        "fusw": fusw.astype(BF),
        "fusb": fusb.astype(BF),
        "i128f": np.eye(128, dtype=np.float32),
        "i128b": np.eye(128).astype(BF),
    }
    if has_ln:
        shared["lng"] = np.stack(
            [np.broadcast_to(g, (128, G)) for g in lngs], axis=1
        ).astype(np.float32).copy()
        shared["lnb"] = np.stack(
            [np.broadcast_to(bb, (128, G)) for bb in lnbs], axis=1
        ).astype(np.float32).copy()

    if pos_same:
        shared["pos0"] = build_pos0(pidx)[None].astype(np.float32)
        shared["pos1"] = build_pos1(pidx)[None].astype(BF)
        pos_per_b = False
    else:
        pos_per_b = True

    in_maps = []
    for c in range(NCORES):
        sl = slice(c * BL, (c + 1) * BL)
        m = dict(shared)
        m["hT"] = hT[sl].astype(BF)
        m["negms"] = negms[sl]
        m["negmm"] = negmm[sl]
        if pos_per_b:
            m["pos0"] = np.stack([build_pos0(positions[i]) for i in range(c * BL, (c + 1) * BL)]).astype(np.float32)
            m["pos1"] = np.stack([build_pos1(positions[i]) for i in range(c * BL, (c + 1) * BL)]).astype(BF)
        in_maps.append(m)

    flags = (BL, pos_per_b, has_tb, has_ln, has_fusb)
    return in_maps, flags


def _get_program(flags):
    if flags not in _prog_cache:
        _prog_cache[flags] = _build_program(*flags)
    return _prog_cache[flags]


_last_results = {}


def kernel(**inputs):
    in_maps, flags = _host_pack(inputs)
    nc = _get_program(flags)
    res = run_bass_kernel_spmd(nc, in_maps, list(range(NCORES)))
    _last_results["res"] = res
    out = np.concatenate([res.results[c]["out"] for c in range(NCORES)], axis=0)
    return np.ascontiguousarray(out.astype(np.float32))
